# revision 1
# baseline (speedup 1.0000x reference)
"""Trainium2 Bass kernel for AttentionWithCAE.

Reference computation (B=8, N=1024, C=768, H=12, hd=64):
    qkv  = x @ qkv_w.T + concat(q_bias, 0, v_bias)
    q,k,v per head; attn = softmax(mask(q*scale @ k.T)); out = attn @ v
    final = out @ proj_w.T + proj_b

Sharding: pure data parallel — batch b on core b, weights replicated,
no collectives.

Device-side layout strategy (per core):
  - Host pre-transposes operands so the device kernel does zero transposes:
      xT [C, N], wqkT [C, 3C] (q-cols pre-scaled by SCALE), pwT [C, C],
      all cast to bf16 on the host (PSUM accumulation stays fp32).
  - qk projection emitted as qkT [1536, N] (feature-major): head h's qT/kT
    are rows h*64..h*64+64 — exactly the lhsT/rhs layout the scores matmul
    needs (contraction over head_dim).
  - v projection emitted token-major [N, 768] interleaved into v65 tiles
    [128, 12*65]: per head 64 v-columns plus a baked ones column, so one
    M=65 matmul per (head, k-tile, q-chunk) yields both attn@v and the
    softmax denominators (row 64 of PSUM).
  - scores computed transposed [k, q]: the key-dependent mask bias becomes a
    per-partition bias folded into the Exp activation (single ACT op;
    no max-subtraction needed: |scores| <= ~10 so exp can't overflow).
  - softmax denominators -> SBUF -> approx reciprocal -> partition-broadcast
    via a DRAM bounce (DMA broadcast needs a DRAM source).
  - attn output accumulates transposed [hd, t] which directly feeds the
    proj matmul; final output is [C, N] and the host transposes it back.
  - q_bias folds into the qkT eviction (per-partition bias); v_bias folds
    into an effective proj bias on the host (attn rows sum to 1).

Scheduling (the emission order shapes the per-engine execution order):
  - v-projection first, then per head-pair p: its two qkT tiles, then the
    pair's scores (row-packed: even head rows 0-63, odd head rows 64-127 ->
    concurrent K=64 matmuls), with the PREVIOUS pair's attn@v matmuls
    interleaved kt-by-kt. QKV work for pair p+1 fills PE gaps while ACT
    runs the exps of pair p, keeping the PE dense (no HAM re-throttle).
"""

import sys

sys.path.insert(0, "/opt/trn_rl_repo")

from contextlib import ExitStack

import numpy as np
import ml_dtypes

import concourse.bass as bass
import concourse.bacc as bacc
import concourse.mybir as mybir
from concourse import tile
from concourse.bass_utils import run_bass_kernel_spmd

B, N, C = 8, 1024, 768
H, HD = 12, 64
F3 = 3 * C  # 2304
SCALE = HD ** -0.5
F32 = mybir.dt.float32
BF16 = mybir.dt.bfloat16
Act = mybir.ActivationFunctionType

MASK_NEG = -30000.0

CT = C // 128  # 6 contraction tiles
TT = N // 128  # 8 token tiles
QKT = 2 * C // 128  # 12 qk feature tiles
NPAIR = H // 2  # 6 head pairs

_CACHE = {}


def _build_nc():
    nc = bacc.Bacc(None, target_bir_lowering=False)

    xT_d = nc.declare_dram_parameter("xT", [C, N], BF16, isOutput=False)
    wqk_d = nc.declare_dram_parameter("wqkT", [C, F3], BF16, isOutput=False)
    pw_d = nc.declare_dram_parameter("pwT", [C, C], BF16, isOutput=False)
    qkb_d = nc.declare_dram_parameter("qkb", [2 * C], F32, isOutput=False)
    mb_d = nc.declare_dram_parameter("mb", [N], F32, isOutput=False)
    pb_d = nc.declare_dram_parameter("pb", [C], F32, isOutput=False)
    out_d = nc.declare_dram_parameter("out", [C, N], F32, isOutput=True)

    r_d = nc.dram_tensor("r_scratch", [H, N], F32)

    with ExitStack() as ctx:
        tc = ctx.enter_context(tile.TileContext(nc))
        pool = ctx.enter_context(tc.tile_pool(name="main", bufs=1))
        psum = ctx.enter_context(tc.tile_pool(name="psum", bufs=1, space="PSUM"))

        qkb_sb = pool.tile([128, QKT], F32)
        nc.sync.dma_start(out=qkb_sb, in_=qkb_d.rearrange("(i p) -> p i", p=128))
        mb_sb = pool.tile([128, TT], F32)
        nc.sync.dma_start(out=mb_sb, in_=mb_d.rearrange("(i p) -> p i", p=128))
        pb_sb = pool.tile([128, CT], F32)
        nc.sync.dma_start(out=pb_sb, in_=pb_d.rearrange("(i p) -> p i", p=128))

        wqk = []
        xTs = []
        for c in range(CT):
            w = pool.tile([128, F3], BF16, tag="wqk", bufs=CT, name=f"wqk{c}")
            for j in range(2):
                nc.sync.dma_start(
                    out=w[:, j * 1152 : (j + 1) * 1152],
                    in_=wqk_d[c * 128 : (c + 1) * 128, j * 1152 : (j + 1) * 1152],
                )
            wqk.append(w)
            xt = pool.tile([128, N], BF16, tag="xT", bufs=CT, name=f"xT{c}")
            nc.sync.dma_start(out=xt, in_=xT_d[c * 128 : (c + 1) * 128, :])
            xTs.append(xt)
        pw = []
        for c in range(CT):
            w = pool.tile([128, C], BF16, tag="pw", bufs=CT, name=f"pw{c}")
            nc.sync.dma_start(out=w, in_=pw_d[c * 128 : (c + 1) * 128, :])
            pw.append(w)

        qkT = [
            pool.tile([128, N], BF16, tag="qkT", bufs=QKT, name=f"qkT{i}")
            for i in range(QKT)
        ]
        v65 = [
            pool.tile([128, H * 65], BF16, tag="v65", bufs=TT, name=f"v65_{i}")
            for i in range(TT)
        ]
        aoT = [
            pool.tile([128, N], BF16, tag="aoT", bufs=CT, name=f"aoT{i}")
            for i in range(CT)
        ]

        def emit_v_tile(ti):
            psa = psum.tile(
                [128, 512], F32, tag=f"psAV{ti % 2}0", bufs=1, name=f"ps_va{ti}"
            )
            psb = psum.tile(
                [128, 256], F32, tag=f"psAV{ti % 2}1", bufs=1, name=f"ps_vb{ti}"
            )
            for c in range(CT):
                nc.tensor.matmul(
                    psa,
                    lhsT=xTs[c][:, ti * 128 : (ti + 1) * 128],
                    rhs=wqk[c][:, 1536:2048],
                    start=(c == 0),
                    stop=(c == CT - 1),
                )
                nc.tensor.matmul(
                    psb,
                    lhsT=xTs[c][:, ti * 128 : (ti + 1) * 128],
                    rhs=wqk[c][:, 2048:2304],
                    start=(c == 0),
                    stop=(c == CT - 1),
                )
            v3 = v65[ti].rearrange("p (h j) -> p h j", j=65)
            nc.scalar.activation(
                v3[:, 0:8, 0:64], psa.rearrange("p (h j) -> p h j", j=64), Act.Copy
            )
            nc.scalar.activation(
                v3[:, 8:12, 0:64], psb.rearrange("p (h j) -> p h j", j=64), Act.Copy
            )
            nc.vector.memset(v3[:, :, 64:65], 1.0)

        def emit_qk_tile(fi):
            ps = psum.tile([128, N], F32, tag="psA", bufs=2, name=f"ps_qk{fi}")
            for c in range(CT):
                for qc in range(2):
                    nc.tensor.matmul(
                        ps[:, qc * 512 : (qc + 1) * 512],
                        lhsT=wqk[c][:, fi * 128 : (fi + 1) * 128],
                        rhs=xTs[c][:, qc * 512 : (qc + 1) * 512],
                        start=(c == 0),
                        stop=(c == CT - 1),
                    )
            nc.vector.tensor_scalar_add(
                out=qkT[fi], in0=ps, scalar1=qkb_sb[:, fi : fi + 1]
            )

        def emit_av_kt(pr, kt):
            for hi, (h, atiles) in enumerate(
                [(pr["h0"], pr["at0"]), (pr["h1"], pr["at1"])]
            ):
                for qc in range(2):
                    nc.tensor.matmul(
                        pr["pav"][hi][qc][0:65, :],
                        lhsT=v65[kt][:, h * 65 : (h + 1) * 65],
                        rhs=atiles[kt][:, qc * 512 : (qc + 1) * 512],
                        start=(kt == 0),
                        stop=(kt == TT - 1),
                    )

        def finish_pair(pr):
            # Evict AV PSUM to SBUF right away (fast DVE copies release the
            # PSUM banks so the next pair's AV can start), then run the slow
            # normalization chain (recip -> DRAM-bounce broadcast -> mul)
            # entirely from SBUF, off the PE stream.
            for hi, h in enumerate([pr["h0"], pr["h1"]]):
                qt, row = h // 2, (h % 2) * 64
                pav = pr["pav"][hi]
                un = [
                    pool.tile([64, 512], F32, tag=f"un{qc}", bufs=2, name=f"un{h}_{qc}")
                    for qc in range(2)
                ]
                srow = pool.tile([1, N], F32, tag="srow", bufs=2, name=f"s{h}")
                for qc in range(2):
                    nc.vector.tensor_copy(out=un[qc], in_=pav[qc][0:64, :])
                    nc.vector.tensor_copy(
                        out=srow[:, qc * 512 : (qc + 1) * 512], in_=pav[qc][64:65, :]
                    )
                r_row = pool.tile([1, N], F32, tag="rrow", bufs=2, name=f"r{h}")
                nc.vector.reciprocal_approx_fast(out=r_row, in_=srow)
                nc.sync.dma_start(out=r_d[h : h + 1, :], in_=r_row)
                r2 = pool.tile([64, N], F32, tag="r2", bufs=3, name=f"r2_{h}")
                nc.sync.dma_start(out=r2, in_=r_d[h : h + 1, :].to_broadcast([64, N]))
                for qc in range(2):
                    nc.vector.tensor_mul(
                        out=aoT[qt][row : row + 64, qc * 512 : (qc + 1) * 512],
                        in0=un[qc][0:64, :],
                        in1=r2[:, qc * 512 : (qc + 1) * 512],
                    )

        prev = None
        for p in range(NPAIR):
            emit_qk_tile(p)
            emit_qk_tile(CT + p)
            h0, h1 = 2 * p, 2 * p + 1
            k_tile, q_tile = qkT[CT + p], qkT[p]
            at0, at1 = [], []
            pav = []
            for hi in range(2):
                row = [
                    psum.tile(
                        [128, 512],
                        F32,
                        tag=f"psAV{hi}{qc}",
                        bufs=1,
                        name=f"pav{2 * p + hi}_{qc}",
                    )
                    for qc in range(2)
                ]
                pav.append(row)
            cur = {"h0": h0, "h1": h1, "at0": at0, "at1": at1, "pav": pav}
            for kt in range(TT):
                ps0 = psum.tile([128, N], F32, tag="psA", bufs=2, name=f"ps_s{h0}_{kt}")
                ps1 = psum.tile([128, N], F32, tag="psA", bufs=2, name=f"ps_s{h1}_{kt}")
                for qc in range(2):
                    # row-packed pair: even head rows 0-63, odd head rows 64-127
                    nc.tensor.matmul(
                        ps0[:, qc * 512 : (qc + 1) * 512],
                        lhsT=k_tile[0:64, kt * 128 : (kt + 1) * 128],
                        rhs=q_tile[0:64, qc * 512 : (qc + 1) * 512],
                        start=True,
                        stop=True,
                    )
                    nc.tensor.matmul(
                        ps1[:, qc * 512 : (qc + 1) * 512],
                        lhsT=k_tile[64:128, kt * 128 : (kt + 1) * 128],
                        rhs=q_tile[64:128, qc * 512 : (qc + 1) * 512],
                        start=True,
                        stop=True,
                    )
                if prev is not None:
                    emit_av_kt(prev, kt)
                if p == 0:
                    # pair 0 has no previous pair's AV to fill PE gaps --
                    # interleave the v-projection (needed from pair 1 on)
                    emit_v_tile(kt)
                a0 = pool.tile([128, N], BF16, tag="attn", bufs=26, name=f"at{h0}_{kt}")
                nc.scalar.activation(a0, ps0, Act.Exp, bias=mb_sb[:, kt : kt + 1])
                at0.append(a0)
                a1 = pool.tile([128, N], BF16, tag="attn", bufs=26, name=f"at{h1}_{kt}")
                nc.scalar.activation(a1, ps1, Act.Exp, bias=mb_sb[:, kt : kt + 1])
                at1.append(a1)
            if prev is not None:
                finish_pair(prev)
            prev = cur

        # drain: AV + normalization for the last pair
        for kt in range(TT):
            emit_av_kt(prev, kt)
        finish_pair(prev)

        # warm-keeper: dependency-free matmuls keep the PE busy (and the HAM
        # clock-gate open) while the last normalization chain runs, so proj
        # starts at full clock. Results are never read.
        for wi in range(24):
            wps = psum.tile([128, 512], F32, tag="psAV00", bufs=1, name=f"warm{wi}")
            nc.tensor.matmul(
                wps,
                lhsT=wqk[wi % CT][:, 0:128],
                rhs=xTs[wi % CT][:, 0:512],
                start=True,
                stop=True,
            )

        # ---------------- proj ----------------
        for ot in range(CT):
            ps = psum.tile([128, N], F32, tag="psA", bufs=2, name=f"ps_p{ot}")
            for c in range(CT):
                for qc in range(2):
                    nc.tensor.matmul(
                        ps[:, qc * 512 : (qc + 1) * 512],
                        lhsT=pw[c][:, ot * 128 : (ot + 1) * 128],
                        rhs=aoT[c][:, qc * 512 : (qc + 1) * 512],
                        start=(c == 0),
                        stop=(c == CT - 1),
                    )
            osb = pool.tile([128, N], F32, tag="osb", bufs=2, name=f"o{ot}")
            nc.scalar.activation(osb, ps, Act.Identity, bias=pb_sb[:, ot : ot + 1])
            nc.sync.dma_start(out=out_d[ot * 128 : (ot + 1) * 128, :], in_=osb)

    nc.finalize()
    return nc


def kernel(x, mask, qkv_w, q_bias, v_bias, proj_w, proj_b, **_):
    x = np.asarray(x, np.float32)
    mask = np.asarray(mask)
    qkv_w = np.asarray(qkv_w, np.float32)
    q_bias = np.asarray(q_bias, np.float32)
    v_bias = np.asarray(v_bias, np.float32)
    proj_w = np.asarray(proj_w, np.float32)
    proj_b = np.asarray(proj_b, np.float32)

    wqkT = np.ascontiguousarray(qkv_w.T)  # [C, 3C]
    wqkT[:, :C] *= SCALE
    qkb = np.concatenate([q_bias * SCALE, np.zeros(C, np.float32)])
    pb_eff = (proj_b + proj_w @ v_bias).astype(np.float32)
    pwT = np.ascontiguousarray(proj_w.T)
    wqkT_bf = wqkT.astype(ml_dtypes.bfloat16)
    pwT_bf = pwT.astype(ml_dtypes.bfloat16)
    mb = np.where(mask, np.float32(MASK_NEG), np.float32(0.0)).astype(np.float32)

    if "nc" not in _CACHE:
        _CACHE["nc"] = _build_nc()
    nc = _CACHE["nc"]

    in_maps = []
    for b in range(B):
        in_maps.append(
            {
                "xT": np.ascontiguousarray(x[b].T).astype(ml_dtypes.bfloat16),
                "wqkT": wqkT_bf,
                "pwT": pwT_bf,
                "qkb": qkb,
                "mb": np.ascontiguousarray(mb[b]),
                "pb": pb_eff,
            }
        )

    _CACHE["last_in_maps"] = in_maps
    res = run_bass_kernel_spmd(nc, in_maps, list(range(B)))
    out = np.stack([res.results[b]["out"].T for b in range(B)], axis=0)
    return out.astype(np.float32)


if __name__ == "__main__":
    np.random.seed(0)
    x = np.random.randn(B, N, C).astype(np.float32)
    mask = np.random.randint(0, 2, (B, N)) > 0
    qkv_w = (np.random.randn(F3, C) * 0.02).astype(np.float32)
    q_bias = (np.random.randn(C) * 0.02).astype(np.float32)
    v_bias = (np.random.randn(C) * 0.02).astype(np.float32)
    proj_w = (np.random.randn(C, C) * 0.02).astype(np.float32)
    proj_b = (np.random.randn(C) * 0.02).astype(np.float32)
    out = kernel(x, mask, qkv_w, q_bias, v_bias, proj_w, proj_b)
    print(out.shape, out.dtype)



# revision 3
# speedup vs baseline: 1.3392x; 1.3392x over previous
"""Trainium2 Bass kernel for AttentionWithCAE.

Reference computation (B=8, N=1024, C=768, H=12, hd=64):
    qkv  = x @ qkv_w.T + concat(q_bias, 0, v_bias)
    q,k,v per head; attn = softmax(mask(q*scale @ k.T)); out = attn @ v
    final = out @ proj_w.T + proj_b

Sharding: pure data parallel -- batch b on core b, weights replicated,
no collectives.

Key structural idea vs the straightforward version: the mask removes
~50% of the KEYS (True = masked out; queries are never masked).  The
host gathers only the unmasked key tokens per batch and pads to a fixed
NK=640 (max real count over the 8 batches is 530), so scores, exp, attn@v
and the k/v projections all shrink by NK/N = 0.625.  This is exact math:
softmax is permutation-invariant over keys and masked keys contribute
exp(-inf) = 0.  Padding keys use zeroed x columns plus the -30000 mask
bias so their exp underflows to 0 (and the baked ones-column contributes
0 to the denominators because the attn weight is 0).

Device-side layout (per core):
  - Host pre-transposes: xT [C,N] (queries side), xkT [C,NK] (gathered
    keys side), wqkT [C,3C] (q cols pre-scaled by SCALE), pwT [C,C],
    all bf16 (PSUM accumulation stays fp32).
  - q projection emitted feature-major qT[f] = [128, N] (f = head pair),
    k projection kT[f] = [128, NK]; head h's qT/kT live in rows
    (h%2)*64..+64 of tile f=h//2 -- exactly the lhsT/rhs layout the
    scores matmul needs (contraction over head_dim).
  - v projection emitted token-major interleaved into v65 tiles
    [128, 12*65]: per head 64 v-columns plus a baked ones column, so the
    M=65 attn@v matmul also yields the softmax denominators (row 64).
  - scores computed transposed [k, q]; per (pair, kt) the two heads of
    the pair go side by side into ONE [128, 1024] PSUM tile per q-chunk:
    tileA = [h0 q0:512 | h1 q0:512], tileB = [h0 q512: | h1 q512:].
    The two matmuls filling a tile use K=64 contraction rows 0:64 /
    64:128 (row-group tiling) and are emitted back-to-back, so they run
    CONCURRENTLY on the PE array (2x on the scores stream).  One exp
    ACTIVATE per tile covers both heads since the mask bias is
    per-partition (key position) and identical for both heads.
  - softmax denominators (row 64 of the AV PSUM) go PSUM -> DRAM,
    come back batched [4, 512] per pair, one reciprocal_approx_fast,
    back to DRAM, then partition-broadcast DMAs [64, 512].
  - attn output accumulates transposed [hd, t] in aoT which directly
    feeds the proj matmul; final output is [C, N], host transposes back.
  - q_bias folds into the qT eviction (per-partition bias); k bias is 0
    (CAE); v_bias folds into an effective proj bias on the host.

Scheduling (emission order == per-engine execution order):
  - Attention phase is ACT(exp)-paced: per (pair, kt) ACT does 2x
    [128,1024] exps (~2.3us) while the PE does 2 concurrent-pair score
    MMs + 4 AV MMs of the previous pair (~1.3us).  The spare PE slots
    are filled by the NEXT pair's q/k projection (rotation slots at
    kt==1 / kt==3) and, during pair 0, the v projection (which borrows
    the idle AV PSUM banks).
  - DMA order front-loads exactly what the first q/k tiles need so the
    first exp fires at ~9us instead of ~18us.
  - proj runs at the tail; its c<5 matmuls don't depend on the last
    pair's normalization, so the PE never goes idle there.
"""

import sys

sys.path.insert(0, "/opt/trn_rl_repo")

from contextlib import ExitStack

import numpy as np
import ml_dtypes

import concourse.bass as bass
import concourse.bacc as bacc
import concourse.mybir as mybir
from concourse import tile
from concourse.bass_utils import run_bass_kernel_spmd

B, N, C = 8, 1024, 768
H, HD = 12, 64
F3 = 3 * C  # 2304
SCALE = HD ** -0.5
F32 = mybir.dt.float32
BF16 = mybir.dt.bfloat16
Act = mybir.ActivationFunctionType

MASK_NEG = -30000.0

NK = 640  # padded unmasked-key count (max real count is ~530)
NKT = NK // 128  # 5 key tiles
CT = C // 128  # 6 contraction tiles
NPAIR = H // 2  # 6 head pairs

_CACHE = {}


def _build_nc():
    nc = bacc.Bacc(None, target_bir_lowering=False)

    xT_d = nc.declare_dram_parameter("xT", [C, N], BF16, isOutput=False)
    xk_d = nc.declare_dram_parameter("xkT", [C, NK], BF16, isOutput=False)
    wqk_d = nc.declare_dram_parameter("wqkT", [C, F3], BF16, isOutput=False)
    pw_d = nc.declare_dram_parameter("pwT", [C, C], BF16, isOutput=False)
    qb_d = nc.declare_dram_parameter("qb", [C], F32, isOutput=False)
    mb_d = nc.declare_dram_parameter("mb", [NK], F32, isOutput=False)
    pb_d = nc.declare_dram_parameter("pb", [C], F32, isOutput=False)
    out_d = nc.declare_dram_parameter("out", [C, N], F32, isOutput=True)

    d_d = nc.dram_tensor("d_scratch", [4 * NPAIR, 512], F32)  # denominators
    r_d = nc.dram_tensor("r_scratch", [4 * NPAIR, 512], F32)  # reciprocals

    with ExitStack() as ctx:
        tc = ctx.enter_context(tile.TileContext(nc))
        pool = ctx.enter_context(tc.tile_pool(name="main", bufs=1))
        psum = ctx.enter_context(tc.tile_pool(name="psum", bufs=1, space="PSUM"))

        # preload the exp table set while DMAs run (no data dependency)
        tiny = pool.tile([1, 8], F32)
        nc.vector.memset(tiny, 0.0)
        tiny2 = pool.tile([1, 8], F32)
        nc.scalar.activation(tiny2, tiny, Act.Exp)

        qb_sb = pool.tile([128, CT], F32)
        nc.sync.dma_start(out=qb_sb, in_=qb_d.rearrange("(i p) -> p i", p=128))
        mb_sb = pool.tile([128, NKT], F32)
        nc.sync.dma_start(out=mb_sb, in_=mb_d.rearrange("(i p) -> p i", p=128))
        pb_sb = pool.tile([128, CT], F32)
        nc.sync.dma_start(out=pb_sb, in_=pb_d.rearrange("(i p) -> p i", p=128))

        # SBUF weight/activation tiles (DMAs emitted in need order below)
        xTs = [
            pool.tile([128, N], BF16, tag="xT", bufs=CT, name=f"xT{c}")
            for c in range(CT)
        ]
        xks = [
            pool.tile([128, NK], BF16, tag="xkT", bufs=CT, name=f"xkT{c}")
            for c in range(CT)
        ]
        wqk = [
            pool.tile([128, F3], BF16, tag="wqk", bufs=CT, name=f"wqk{c}")
            for c in range(CT)
        ]
        pw = [
            pool.tile([128, C], BF16, tag="pw", bufs=CT, name=f"pw{c}")
            for c in range(CT)
        ]

        # DMA order = earliest-need order.
        for c in range(CT):
            nc.sync.dma_start(out=xTs[c], in_=xT_d[c * 128 : (c + 1) * 128, :])
        for c in range(CT):  # q weights, feature tile 0 only
            nc.sync.dma_start(
                out=wqk[c][:, 0:128], in_=wqk_d[c * 128 : (c + 1) * 128, 0:128]
            )
        for c in range(CT):
            nc.sync.dma_start(out=xks[c], in_=xk_d[c * 128 : (c + 1) * 128, :])
        for c in range(CT):  # k weights, feature tile 0 only
            nc.sync.dma_start(
                out=wqk[c][:, 768:896], in_=wqk_d[c * 128 : (c + 1) * 128, 768:896]
            )
        for c in range(CT):  # v weights
            nc.sync.dma_start(
                out=wqk[c][:, 1536:2304],
                in_=wqk_d[c * 128 : (c + 1) * 128, 1536:2304],
            )
        for c in range(CT):  # remaining q weights
            nc.sync.dma_start(
                out=wqk[c][:, 128:768], in_=wqk_d[c * 128 : (c + 1) * 128, 128:768]
            )
        for c in range(CT):  # remaining k weights
            nc.sync.dma_start(
                out=wqk[c][:, 896:1536], in_=wqk_d[c * 128 : (c + 1) * 128, 896:1536]
            )
        for c in range(CT):
            nc.sync.dma_start(out=pw[c], in_=pw_d[c * 128 : (c + 1) * 128, :])

        qT = [
            pool.tile([128, N], BF16, tag="qT", bufs=CT, name=f"qT{f}")
            for f in range(CT)
        ]
        kT = [
            pool.tile([128, NK], BF16, tag="kT", bufs=CT, name=f"kT{f}")
            for f in range(CT)
        ]
        v65 = [
            pool.tile([128, H * 65], BF16, tag="v65", bufs=NKT, name=f"v65_{i}")
            for i in range(NKT)
        ]
        aoT = [
            pool.tile([128, N], BF16, tag="aoT", bufs=CT, name=f"aoT{i}")
            for i in range(CT)
        ]

        def emit_q_tile(f):
            ps = psum.tile([128, N], F32, tag="psS", bufs=2, name=f"ps_q{f}")
            for c in range(CT):
                for qc in range(2):
                    nc.tensor.matmul(
                        ps[:, qc * 512 : (qc + 1) * 512],
                        lhsT=wqk[c][:, f * 128 : (f + 1) * 128],
                        rhs=xTs[c][:, qc * 512 : (qc + 1) * 512],
                        start=(c == 0),
                        stop=(c == CT - 1),
                    )
            nc.vector.tensor_scalar_add(
                out=qT[f], in0=ps, scalar1=qb_sb[:, f : f + 1]
            )

        def emit_k_tile(f):
            ps = psum.tile([128, NK], F32, tag="psS", bufs=2, name=f"ps_k{f}")
            for c in range(CT):
                nc.tensor.matmul(
                    ps[:, 0:512],
                    lhsT=wqk[c][:, 768 + f * 128 : 768 + (f + 1) * 128],
                    rhs=xks[c][:, 0:512],
                    start=(c == 0),
                    stop=(c == CT - 1),
                )
                nc.tensor.matmul(
                    ps[:, 512:640],
                    lhsT=wqk[c][:, 768 + f * 128 : 768 + (f + 1) * 128],
                    rhs=xks[c][:, 512:640],
                    start=(c == 0),
                    stop=(c == CT - 1),
                )
            nc.vector.tensor_copy(out=kT[f], in_=ps)

        def emit_v_tile(ti):
            psa = psum.tile(
                [128, 512], F32, tag=f"psAV{ti % 2}0", bufs=1, name=f"ps_va{ti}"
            )
            psb = psum.tile(
                [128, 256], F32, tag=f"psAV{ti % 2}1", bufs=1, name=f"ps_vb{ti}"
            )
            for c in range(CT):
                nc.tensor.matmul(
                    psa,
                    lhsT=xks[c][:, ti * 128 : (ti + 1) * 128],
                    rhs=wqk[c][:, 1536:2048],
                    start=(c == 0),
                    stop=(c == CT - 1),
                )
                nc.tensor.matmul(
                    psb,
                    lhsT=xks[c][:, ti * 128 : (ti + 1) * 128],
                    rhs=wqk[c][:, 2048:2304],
                    start=(c == 0),
                    stop=(c == CT - 1),
                )
            v3 = v65[ti].rearrange("p (h j) -> p h j", j=65)
            nc.vector.tensor_copy(
                out=v3[:, 0:8, 0:64], in_=psa.rearrange("p (h j) -> p h j", j=64)
            )
            nc.vector.tensor_copy(
                out=v3[:, 8:12, 0:64], in_=psb.rearrange("p (h j) -> p h j", j=64)
            )
            nc.vector.memset(v3[:, :, 64:65], 1.0)

        def emit_av_kt(pr, kt):
            for hi in range(2):
                h = pr["h0"] + hi
                for qc in range(2):
                    a = pr["aA"][kt] if qc == 0 else pr["aB"][kt]
                    nc.tensor.matmul(
                        pr["pav"][hi][qc][0:65, :],
                        lhsT=v65[kt][:, h * 65 : (h + 1) * 65],
                        rhs=a[:, hi * 512 : (hi + 1) * 512],
                        start=(kt == 0),
                        stop=(kt == NKT - 1),
                    )

        def finish_pair(pr):
            # Evict AV PSUM (bf16) fast to release banks, push denominators
            # to DRAM, batch-reciprocal per pair, broadcast back, normalize.
            p = pr["p"]
            un = {}
            for hi in range(2):
                for qc in range(2):
                    j = hi * 2 + qc
                    u = pool.tile(
                        [65, 512], F32, tag=f"un{j}", bufs=2, name=f"un{p}_{j}"
                    )
                    nc.vector.tensor_copy(out=u, in_=pr["pav"][hi][qc][0:65, :])
                    un[j] = u
                    nc.sync.dma_start(
                        out=d_d[4 * p + j : 4 * p + j + 1, :],
                        in_=u[64:65, :],
                    )
            dsb = pool.tile([4, 512], F32, tag="dsb", bufs=2, name=f"dsb{p}")
            nc.sync.dma_start(out=dsb, in_=d_d[4 * p : 4 * p + 4, :])
            rsb = pool.tile([4, 512], F32, tag="rsb", bufs=2, name=f"rsb{p}")
            nc.vector.reciprocal_approx_fast(out=rsb, in_=dsb)
            nc.sync.dma_start(out=r_d[4 * p : 4 * p + 4, :], in_=rsb)
            for hi in range(2):
                for qc in range(2):
                    j = hi * 2 + qc
                    r2 = pool.tile(
                        [64, 512], F32, tag="r2", bufs=4, name=f"r2_{p}_{j}"
                    )
                    nc.sync.dma_start(
                        out=r2,
                        in_=r_d[4 * p + j : 4 * p + j + 1, :].to_broadcast([64, 512]),
                    )
                    nc.vector.tensor_mul(
                        out=aoT[p][hi * 64 : (hi + 1) * 64, qc * 512 : (qc + 1) * 512],
                        in0=un[j][0:64, :],
                        in1=r2,
                    )

        emit_q_tile(0)
        emit_k_tile(0)

        prev = None
        for p in range(NPAIR):
            h0 = 2 * p
            pav = []
            for hi in range(2):
                row = [
                    psum.tile(
                        [128, 512],
                        F32,
                        tag=f"psAV{hi}{qc}",
                        bufs=1,
                        name=f"pav{h0 + hi}_{qc}",
                    )
                    for qc in range(2)
                ]
                pav.append(row)
            cur = {"p": p, "h0": h0, "aA": [], "aB": [], "pav": pav}
            for kt in range(NKT):
                sA = psum.tile([128, N], F32, tag="psS", bufs=2, name=f"sA{p}_{kt}")
                sB = psum.tile([128, N], F32, tag="psS", bufs=2, name=f"sB{p}_{kt}")
                for hi in range(2):
                    base = hi * 64
                    nc.tensor.matmul(
                        sA[:, hi * 512 : (hi + 1) * 512],
                        lhsT=kT[p][base : base + 64, kt * 128 : (kt + 1) * 128],
                        rhs=qT[p][base : base + 64, 0:512],
                        start=True,
                        stop=True,
                    )
                for hi in range(2):
                    base = hi * 64
                    nc.tensor.matmul(
                        sB[:, hi * 512 : (hi + 1) * 512],
                        lhsT=kT[p][base : base + 64, kt * 128 : (kt + 1) * 128],
                        rhs=qT[p][base : base + 64, 512:1024],
                        start=True,
                        stop=True,
                    )
                if prev is not None:
                    emit_av_kt(prev, kt)
                if p == 0:
                    emit_v_tile(kt)
                if p + 1 < NPAIR:
                    if kt == 1:
                        emit_q_tile(p + 1)
                    elif kt == 3:
                        emit_k_tile(p + 1)
                aA = pool.tile([128, N], BF16, tag="attn", bufs=22, name=f"aA{p}_{kt}")
                nc.scalar.activation(aA, sA, Act.Exp, bias=mb_sb[:, kt : kt + 1])
                cur["aA"].append(aA)
                aB = pool.tile([128, N], BF16, tag="attn", bufs=22, name=f"aB{p}_{kt}")
                nc.scalar.activation(aB, sB, Act.Exp, bias=mb_sb[:, kt : kt + 1])
                cur["aB"].append(aB)
            if prev is not None:
                finish_pair(prev)
            prev = cur

        # drain: AV + normalization for the last pair
        for kt in range(NKT):
            emit_av_kt(prev, kt)
        finish_pair(prev)

        # ---------------- proj ----------------
        # c < 5 matmuls only need aoT[0..4] (ready); the c == 5 matmuls are
        # reached in the PE queue well after the last pair's normalization
        # lands, so the PE stays dense through the tail.
        for ot in range(CT):
            ps = psum.tile([128, N], F32, tag="psS", bufs=2, name=f"ps_p{ot}")
            for c in range(CT):
                for qc in range(2):
                    nc.tensor.matmul(
                        ps[:, qc * 512 : (qc + 1) * 512],
                        lhsT=pw[c][:, ot * 128 : (ot + 1) * 128],
                        rhs=aoT[c][:, qc * 512 : (qc + 1) * 512],
                        start=(c == 0),
                        stop=(c == CT - 1),
                    )
            osb = pool.tile([128, N], F32, tag="osb", bufs=2, name=f"o{ot}")
            nc.scalar.activation(osb, ps, Act.Identity, bias=pb_sb[:, ot : ot + 1])
            nc.sync.dma_start(out=out_d[ot * 128 : (ot + 1) * 128, :], in_=osb)

    nc.finalize()
    return nc


def _kernel_numpy(x, mask, qkv_w, q_bias, v_bias, proj_w, proj_b):
    # exact-reference fallback (never hit for the fixed problem inputs)
    qkv_bias = np.concatenate([q_bias, np.zeros_like(v_bias), v_bias])
    out = np.empty_like(x)
    for b in range(x.shape[0]):
        qkv = x[b] @ qkv_w.T + qkv_bias
        qkv = qkv.reshape(N, 3, H, HD)
        q, k, v = (qkv[:, i].transpose(1, 0, 2) for i in range(3))
        attn = np.einsum("hqd,hkd->hqk", q * SCALE, k)
        attn = np.where(mask[b][None, None, :], -np.inf, attn)
        attn = attn - attn.max(axis=-1, keepdims=True)
        e = np.exp(attn)
        attn = e / e.sum(axis=-1, keepdims=True)
        o = np.einsum("hqk,hkd->hqd", attn, v)
        o = o.transpose(1, 0, 2).reshape(N, C)
        out[b] = o @ proj_w.T + proj_b
    return out


def kernel(x, mask, qkv_w, q_bias, v_bias, proj_w, proj_b, **_):
    x = np.asarray(x, np.float32)
    mask = np.asarray(mask)
    qkv_w = np.asarray(qkv_w, np.float32)
    q_bias = np.asarray(q_bias, np.float32)
    v_bias = np.asarray(v_bias, np.float32)
    proj_w = np.asarray(proj_w, np.float32)
    proj_b = np.asarray(proj_b, np.float32)

    if int((~mask).sum(axis=1).max()) > NK:
        return _kernel_numpy(x, mask, qkv_w, q_bias, v_bias, proj_w, proj_b)

    wqkT = np.ascontiguousarray(qkv_w.T)  # [C, 3C]
    wqkT[:, :C] *= SCALE
    qb = (q_bias * SCALE).astype(np.float32)
    pb_eff = (proj_b + proj_w @ v_bias).astype(np.float32)
    pwT = np.ascontiguousarray(proj_w.T)
    wqkT_bf = wqkT.astype(ml_dtypes.bfloat16)
    pwT_bf = pwT.astype(ml_dtypes.bfloat16)

    if "nc" not in _CACHE:
        _CACHE["nc"] = _build_nc()
    nc = _CACHE["nc"]

    in_maps = []
    for b in range(B):
        idx = np.flatnonzero(~mask[b])
        nk = len(idx)
        xkT = np.zeros((C, NK), np.float32)
        xkT[:, :nk] = x[b][idx].T
        mb = np.full(NK, MASK_NEG, np.float32)
        mb[:nk] = 0.0
        in_maps.append(
            {
                "xT": np.ascontiguousarray(x[b].T).astype(ml_dtypes.bfloat16),
                "xkT": xkT.astype(ml_dtypes.bfloat16),
                "wqkT": wqkT_bf,
                "pwT": pwT_bf,
                "qb": qb,
                "mb": mb,
                "pb": pb_eff,
            }
        )

    _CACHE["last_in_maps"] = in_maps
    res = run_bass_kernel_spmd(nc, in_maps, list(range(B)))
    out = np.stack([res.results[b]["out"].T for b in range(B)], axis=0)
    return out.astype(np.float32)


if __name__ == "__main__":
    np.random.seed(0)
    x = np.random.randn(B, N, C).astype(np.float32)
    mask = np.random.randint(0, 2, (B, N)) > 0
    qkv_w = (np.random.randn(F3, C) * 0.02).astype(np.float32)
    q_bias = (np.random.randn(C) * 0.02).astype(np.float32)
    v_bias = (np.random.randn(C) * 0.02).astype(np.float32)
    proj_w = (np.random.randn(C, C) * 0.02).astype(np.float32)
    proj_b = (np.random.randn(C) * 0.02).astype(np.float32)
    out = kernel(x, mask, qkv_w, q_bias, v_bias, proj_w, proj_b)
    ref = _kernel_numpy(x, mask, qkv_w, q_bias, v_bias, proj_w, proj_b)
    rel = np.linalg.norm(out - ref) / np.linalg.norm(ref)
    print(out.shape, out.dtype, "rel err vs numpy:", rel)


# revision 12
# speedup vs baseline: 1.3448x; 1.0042x over previous
"""Trainium2 Bass kernel for AttentionWithCAE.

Reference computation (B=8, N=1024, C=768, H=12, hd=64):
    qkv  = x @ qkv_w.T + concat(q_bias, 0, v_bias)
    q,k,v per head; attn = softmax(mask(q*scale @ k.T)); out = attn @ v
    final = out @ proj_w.T + proj_b

Sharding: pure data parallel -- batch b on core b, weights replicated,
no collectives.

Structural ideas (in rough order of impact):

1. Key gather: the mask removes ~50% of the KEYS (True = masked out;
   queries are never masked).  The host gathers only the unmasked key
   tokens per batch and pads to NK=640 (max real count is 530), so
   scores, exp, attn@v and the k/v projections shrink by 0.625.  Exact:
   softmax is permutation-invariant over keys; padding keys have zeroed
   x columns and a -30000 exp bias so they contribute exactly 0.

2. Scores transposed [k, q] with the two heads of a pair side by side in
   one [128, 1024] PSUM tile per q-chunk (tileA = q0:512, tileB =
   q512:1024; head hi in columns hi*512..).  The two K=64 matmuls
   filling a tile target row groups 0:64 / 64:128 and are emitted
   back-to-back, so they run CONCURRENTLY on the PE array (measured
   dt ~5ns).  One exp ACTIVATE per tile covers both heads (mask bias is
   per-partition = per key, identical for both).

3. ACT (exp) paces the attention phases.  Pair p's attn@v (with the
   baked ones-column giving the softmax denominators as PSUM row 64) is
   DEFERRED into phase p+1 and interleaved kt-by-kt so its matmuls never
   wait on fresh exps; the last AV chunk is pulled one slot early
   (slot 3) so the normalization chain (slot 4) completes by the phase
   boundary and the next pair's AV never stalls on the PSUM banks.

4. Normalization: denominator row -> partition 0 (single-partition
   custom/ISA ops only work at base 0), reciprocal_approx_fast on DVE,
   partition_broadcast on the otherwise-idle GPSIMD, then one DVE
   multiply per (head, q-chunk) STRAIGHT from the AV PSUM into aoT.

5. DMA: descriptor rows stripe across all 16 DMA engines, so few FAT
   DMAs beat many small ones; dispatch costs ~0.7us per dma_start on an
   engine queue, so inputs are host-packed into 8 large tensors (rows
   1.5-12KB), issued round-robin over the sync/scalar/gpsimd queues in
   earliest-need order.  Output is bf16 (host upcasts).

6. HAM warm-up: ~4us of F=512 garbage matmuls during the DMA dead time
   so the PE is at full clock when real work arrives (F=8 matmuls do
   NOT warm it -- only ~5% array duty).

7. Tail: proj is ot-major from the scores PSUM slots; ot0/ot1's c<5
   matmuls are emitted before the c==5 ones so the last pair's
   normalization latency is fully hidden behind real work.
"""

import sys

sys.path.insert(0, "/opt/trn_rl_repo")

from contextlib import ExitStack

import numpy as np
import ml_dtypes

import concourse.bass as bass
import concourse.bacc as bacc
import concourse.mybir as mybir
from concourse import tile
from concourse.bass_utils import run_bass_kernel_spmd

B, N, C = 8, 1024, 768
H, HD = 12, 64
F3 = 3 * C  # 2304
SCALE = HD ** -0.5
F32 = mybir.dt.float32
BF16 = mybir.dt.bfloat16
Act = mybir.ActivationFunctionType

MASK_NEG = -30000.0

NK = 640  # padded unmasked-key count (max real count is ~530)
NKT = NK // 128  # 5 key tiles
CT = C // 128  # 6 contraction tiles
NPAIR = H // 2  # 6 head pairs

_CACHE = {}


def _build_nc():
    nc = bacc.Bacc(None, target_bir_lowering=False)

    # host-packed layouts (see _pack_weights / kernel()):
    #   xTr  [128, CT*1024] queries, c-major:   [p, c*1024 + t]
    #   xkr  [128, CT*640]  gathered keys, c-major
    #   wqk0 [128, 1536]    q/k weights feature-tile 0: [wq_f0 | wk_f0]
    #   wqf  [128, CT*768]  q weights, feature-tile-major
    #   wkf  [128, CT*768]  k weights, feature-tile-major
    #   wvc  [128, CT*768]  v weights, c-major
    #   pwf  [128, CT*768]  proj weights, out-tile-major
    xT_d = nc.declare_dram_parameter("xTr", [128, CT * 1024], BF16, isOutput=False)
    xk_d = nc.declare_dram_parameter("xkr", [128, CT * 640], BF16, isOutput=False)
    wqk0_d = nc.declare_dram_parameter("wqk0", [128, 1536], BF16, isOutput=False)
    wqf_d = nc.declare_dram_parameter("wqf", [128, CT * 768], BF16, isOutput=False)
    wkf_d = nc.declare_dram_parameter("wkf", [128, CT * 768], BF16, isOutput=False)
    wvc_d = nc.declare_dram_parameter("wvc", [128, CT * 768], BF16, isOutput=False)
    pwf_d = nc.declare_dram_parameter("pwf", [128, CT * 768], BF16, isOutput=False)
    qb_d = nc.declare_dram_parameter("qb", [C], F32, isOutput=False)
    mb_d = nc.declare_dram_parameter("mb", [NK], F32, isOutput=False)
    pb_d = nc.declare_dram_parameter("pb", [C], F32, isOutput=False)
    out_d = nc.declare_dram_parameter("out", [C, N], BF16, isOutput=True)

    with ExitStack() as ctx:
        tc = ctx.enter_context(tile.TileContext(nc))
        pool = ctx.enter_context(tc.tile_pool(name="main", bufs=1))
        psum = ctx.enter_context(tc.tile_pool(name="psum", bufs=1, space="PSUM"))

        # preload the exp table set while DMAs run (no data dependency)
        tiny = pool.tile([1, 8], F32)
        nc.vector.memset(tiny, 0.0)
        tiny2 = pool.tile([1, 8], F32)
        nc.scalar.activation(tiny2, tiny, Act.Exp)

        qb_sb = pool.tile([128, CT], F32)
        nc.sync.dma_start(out=qb_sb, in_=qb_d.rearrange("(i p) -> p i", p=128))
        mb_sb = pool.tile([128, NKT], F32)
        nc.scalar.dma_start(out=mb_sb, in_=mb_d.rearrange("(i p) -> p i", p=128))
        pb_sb = pool.tile([128, CT], F32)
        nc.gpsimd.dma_start(out=pb_sb, in_=pb_d.rearrange("(i p) -> p i", p=128))

        # HAM warm-up: full-width F=512 matmuls (high array duty) during
        # the DMA dead time so real work starts at 2.4GHz.
        warm_w = pool.tile([128, 128], BF16, name="warmw")
        nc.vector.memset(warm_w, 0.25)
        warm_x = pool.tile([128, 512], BF16, name="warmx")
        nc.vector.memset(warm_x, 0.25)
        wps = psum.tile([128, 512], F32, tag="psAV00", bufs=1, name="warm")
        for wi in range(22):
            nc.tensor.matmul(wps, lhsT=warm_w, rhs=warm_x, start=True, stop=True)

        xTall = pool.tile([128, CT * 1024], BF16, name="xTall")
        xkall = pool.tile([128, CT * 640], BF16, name="xkall")
        wqk0 = pool.tile([128, 1536], BF16, name="wqk0")
        wqf = pool.tile([128, CT * 768], BF16, name="wqf")
        wkf = pool.tile([128, CT * 768], BF16, name="wkf")
        wvc = pool.tile([128, CT * 768], BF16, name="wvc")
        pwf = pool.tile([128, CT * 768], BF16, name="pwf")

        def xTs(c):
            return xTall[:, c * 1024 : (c + 1) * 1024]

        def xks(c):
            return xkall[:, c * 640 : (c + 1) * 640]

        # input DMAs: few fat transfers, earliest-need first, spread over
        # three dispatch queues.
        nc.sync.dma_start(out=wqk0, in_=wqk0_d[:, :])
        nc.scalar.dma_start(out=xTall, in_=xT_d[:, :])
        nc.gpsimd.dma_start(out=xkall, in_=xk_d[:, :])
        nc.sync.dma_start(out=wvc, in_=wvc_d[:, :])
        nc.scalar.dma_start(out=wqf, in_=wqf_d[:, :])
        nc.gpsimd.dma_start(out=wkf, in_=wkf_d[:, :])
        nc.sync.dma_start(out=pwf, in_=pwf_d[:, :])

        qT = [
            pool.tile([128, N], BF16, tag="qT", bufs=CT, name=f"qT{f}")
            for f in range(CT)
        ]
        kT = [
            pool.tile([128, NK], BF16, tag="kT", bufs=CT, name=f"kT{f}")
            for f in range(CT)
        ]
        v65 = [
            pool.tile([128, H * 65], BF16, tag="v65", bufs=NKT, name=f"v65_{i}")
            for i in range(NKT)
        ]
        aoT = [
            pool.tile([128, N], BF16, tag="aoT", bufs=CT, name=f"aoT{i}")
            for i in range(CT)
        ]

        def q_lhsT(f, c):
            if f == 0:
                return wqk0[:, c * 128 : (c + 1) * 128]
            return wqf[:, f * 768 + c * 128 : f * 768 + (c + 1) * 128]

        def k_lhsT(f, c):
            if f == 0:
                return wqk0[:, 768 + c * 128 : 768 + (c + 1) * 128]
            return wkf[:, f * 768 + c * 128 : f * 768 + (c + 1) * 128]

        def emit_q_half(f, qc):
            ps = psum.tile([128, 512], F32, tag="psS", bufs=2, name=f"ps_q{f}_{qc}")
            sl = slice(qc * 512, (qc + 1) * 512)
            for c in range(CT):
                nc.tensor.matmul(
                    ps,
                    lhsT=q_lhsT(f, c),
                    rhs=xTs(c)[:, sl],
                    start=(c == 0),
                    stop=(c == CT - 1),
                )
            nc.vector.tensor_scalar_add(
                out=qT[f][:, sl], in0=ps, scalar1=qb_sb[:, f : f + 1]
            )

        def emit_k_half(f, kc):
            w = 512 if kc == 0 else 128
            ps = psum.tile([128, w], F32, tag="psS", bufs=2, name=f"ps_k{f}_{kc}")
            sl = slice(0, 512) if kc == 0 else slice(512, 640)
            for c in range(CT):
                nc.tensor.matmul(
                    ps,
                    lhsT=k_lhsT(f, c),
                    rhs=xks(c)[:, sl],
                    start=(c == 0),
                    stop=(c == CT - 1),
                )
            nc.vector.tensor_copy(out=kT[f][:, sl], in_=ps)

        def emit_v_tile(ti):
            psa = psum.tile(
                [128, 512], F32, tag=f"psAV{ti % 2}0", bufs=1, name=f"ps_va{ti}"
            )
            psb = psum.tile(
                [128, 256], F32, tag=f"psAV{ti % 2}1", bufs=1, name=f"ps_vb{ti}"
            )
            for c in range(CT):
                nc.tensor.matmul(
                    psa,
                    lhsT=xks(c)[:, ti * 128 : (ti + 1) * 128],
                    rhs=wvc[:, c * 768 : c * 768 + 512],
                    start=(c == 0),
                    stop=(c == CT - 1),
                )
                nc.tensor.matmul(
                    psb,
                    lhsT=xks(c)[:, ti * 128 : (ti + 1) * 128],
                    rhs=wvc[:, c * 768 + 512 : (c + 1) * 768],
                    start=(c == 0),
                    stop=(c == CT - 1),
                )
            v3 = v65[ti].rearrange("p (h j) -> p h j", j=65)
            nc.vector.tensor_copy(
                out=v3[:, 0:8, 0:64], in_=psa.rearrange("p (h j) -> p h j", j=64)
            )
            nc.vector.tensor_copy(
                out=v3[:, 8:12, 0:64], in_=psb.rearrange("p (h j) -> p h j", j=64)
            )
            nc.vector.memset(v3[:, :, 64:65], 1.0)

        def emit_av_kt(pr, kt):
            for hi in range(2):
                h = pr["h0"] + hi
                for qc in range(2):
                    a = pr["aA"][kt] if qc == 0 else pr["aB"][kt]
                    nc.tensor.matmul(
                        pr["pav"][hi][qc][0:65, :],
                        lhsT=v65[kt][:, h * 65 : (h + 1) * 65],
                        rhs=a[:, hi * 512 : (hi + 1) * 512],
                        start=(kt == 0),
                        stop=(kt == NKT - 1),
                    )

        def finish_pair(pr):
            p = pr["p"]
            for hi in range(2):
                for qc in range(2):
                    j = hi * 2 + qc
                    s = pool.tile([1, 512], F32, tag=f"s{j}", bufs=2, name=f"s{p}_{j}")
                    nc.vector.tensor_copy(out=s, in_=pr["pav"][hi][qc][64:65, :])
                    r1 = pool.tile([1, 512], F32, tag=f"r1{j}", bufs=2, name=f"r1_{p}{j}")
                    nc.vector.reciprocal_approx_fast(out=r1, in_=s)
                    r2 = pool.tile([64, 512], F32, tag=f"r2{j}", bufs=2, name=f"r2_{p}{j}")
                    nc.gpsimd.partition_broadcast(r2, r1)
                    nc.vector.tensor_mul(
                        out=aoT[p][hi * 64 : (hi + 1) * 64, qc * 512 : (qc + 1) * 512],
                        in0=pr["pav"][hi][qc][0:64, :],
                        in1=r2,
                    )

        def emit_scores(p, kt, cur):
            sA = psum.tile([128, N], F32, tag="psS", bufs=2, name=f"sA{p}_{kt}")
            sB = psum.tile([128, N], F32, tag="psS", bufs=2, name=f"sB{p}_{kt}")
            for hi in range(2):
                base = hi * 64
                nc.tensor.matmul(
                    sA[:, hi * 512 : (hi + 1) * 512],
                    lhsT=kT[p][base : base + 64, kt * 128 : (kt + 1) * 128],
                    rhs=qT[p][base : base + 64, 0:512],
                    start=True,
                    stop=True,
                )
            for hi in range(2):
                base = hi * 64
                nc.tensor.matmul(
                    sB[:, hi * 512 : (hi + 1) * 512],
                    lhsT=kT[p][base : base + 64, kt * 128 : (kt + 1) * 128],
                    rhs=qT[p][base : base + 64, 512:1024],
                    start=True,
                    stop=True,
                )
            cur["sA"], cur["sB"] = sA, sB

        def emit_exps(p, kt, cur):
            aA = pool.tile([128, N], BF16, tag="attn", bufs=20, name=f"aA{p}_{kt}")
            nc.scalar.activation(aA, cur["sA"], Act.Exp, bias=mb_sb[:, kt : kt + 1])
            cur["aA"].append(aA)
            aB = pool.tile([128, N], BF16, tag="attn", bufs=20, name=f"aB{p}_{kt}")
            nc.scalar.activation(aB, cur["sB"], Act.Exp, bias=mb_sb[:, kt : kt + 1])
            cur["aB"].append(aB)

        emit_q_half(0, 0)
        emit_k_half(0, 0)
        emit_q_half(0, 1)
        emit_k_half(0, 1)

        prev = None
        for p in range(NPAIR):
            h0 = 2 * p
            pav = []
            for hi in range(2):
                row = [
                    psum.tile(
                        [128, 512],
                        F32,
                        tag=f"psAV{hi}{qc}",
                        bufs=1,
                        name=f"pav{h0 + hi}_{qc}",
                    )
                    for qc in range(2)
                ]
                pav.append(row)
            cur = {"p": p, "h0": h0, "aA": [], "aB": [], "pav": pav}
            for kt in range(NKT):
                emit_scores(p, kt, cur)
                if prev is not None:
                    if kt < NKT - 1:
                        emit_av_kt(prev, kt)
                        if kt == NKT - 2:
                            emit_av_kt(prev, NKT - 1)  # early drain: frees banks
                    else:
                        finish_pair(prev)  # chain completes ~phase boundary
                if p == 0:
                    emit_v_tile(kt)
                if p + 1 < NPAIR:
                    if kt == 1:
                        emit_q_half(p + 1, 0)
                    elif kt == 2:
                        emit_q_half(p + 1, 1)
                    elif kt == 3:
                        emit_k_half(p + 1, 0)
                    elif kt == 4:
                        emit_k_half(p + 1, 1)
                emit_exps(p, kt, cur)
            prev = cur

        # drain: the last pair's AV + normalization
        for kt in range(NKT):
            emit_av_kt(prev, kt)
        finish_pair(prev)

        # ---------------- proj (ot-major, c<5 first for ot0/ot1) ----------
        def proj_mms(ot, ps, c_list):
            for c in c_list:
                for qc in range(2):
                    nc.tensor.matmul(
                        ps[:, qc * 512 : (qc + 1) * 512],
                        lhsT=pwf[:, ot * 768 + c * 128 : ot * 768 + (c + 1) * 128],
                        rhs=aoT[c][:, qc * 512 : (qc + 1) * 512],
                        start=(c == 0),
                        stop=(c == CT - 1),
                    )

        def proj_finish(ot, ps):
            osb = pool.tile([128, N], BF16, tag="osb", bufs=3, name=f"o{ot}")
            nc.scalar.activation(osb, ps, Act.Identity, bias=pb_sb[:, ot : ot + 1])
            eng = [nc.sync, nc.scalar, nc.gpsimd][ot % 3]
            eng.dma_start(out=out_d[ot * 128 : (ot + 1) * 128, :], in_=osb)

        ps0 = psum.tile([128, N], F32, tag="psS", bufs=2, name="ps_p0")
        proj_mms(0, ps0, range(CT - 1))
        ps1 = psum.tile([128, N], F32, tag="psS", bufs=2, name="ps_p1")
        proj_mms(1, ps1, range(CT - 1))
        proj_mms(0, ps0, [CT - 1])
        proj_finish(0, ps0)
        proj_mms(1, ps1, [CT - 1])
        proj_finish(1, ps1)
        for ot in range(2, CT):
            ps = psum.tile([128, N], F32, tag="psS", bufs=2, name=f"ps_p{ot}")
            proj_mms(ot, ps, range(CT))
            proj_finish(ot, ps)

    nc.finalize()
    return nc


def _kernel_numpy(x, mask, qkv_w, q_bias, v_bias, proj_w, proj_b):
    # exact-reference fallback (never hit for the fixed problem inputs)
    qkv_bias = np.concatenate([q_bias, np.zeros_like(v_bias), v_bias])
    out = np.empty_like(x)
    for b in range(x.shape[0]):
        qkv = x[b] @ qkv_w.T + qkv_bias
        qkv = qkv.reshape(N, 3, H, HD)
        q, k, v = (qkv[:, i].transpose(1, 0, 2) for i in range(3))
        attn = np.einsum("hqd,hkd->hqk", q * SCALE, k)
        attn = np.where(mask[b][None, None, :], -np.inf, attn)
        attn = attn - attn.max(axis=-1, keepdims=True)
        e = np.exp(attn)
        attn = e / e.sum(axis=-1, keepdims=True)
        o = np.einsum("hqk,hkd->hqd", attn, v)
        o = o.transpose(1, 0, 2).reshape(N, C)
        out[b] = o @ proj_w.T + proj_b
    return out


def _pack_weights(qkv_w, proj_w):
    wqkT = np.ascontiguousarray(qkv_w.T).astype(np.float32)  # [C, 3C]
    wqkT[:, :C] *= SCALE
    # wqf/wkf: [p, f*768 + c*128 + j] = wqkT[c*128+p, off + f*128 + j]
    wq = wqkT[:, 0:C].reshape(CT, 128, CT, 128)  # [c, p, f, j]
    wqf = np.ascontiguousarray(wq.transpose(1, 2, 0, 3).reshape(128, CT * 768))
    wk = wqkT[:, C : 2 * C].reshape(CT, 128, CT, 128)
    wkf = np.ascontiguousarray(wk.transpose(1, 2, 0, 3).reshape(128, CT * 768))
    wqk0 = np.concatenate([wqf[:, 0:768], wkf[:, 0:768]], axis=1)
    # wvc: [p, c*768 + j] = wqkT[c*128+p, 2C + j]
    wv = wqkT[:, 2 * C : 3 * C].reshape(CT, 128, C)  # [c, p, j]
    wvc = np.ascontiguousarray(wv.transpose(1, 0, 2).reshape(128, CT * 768))
    # pwf: [p, ot*768 + c*128 + j] = proj_w.T[c*128+p, ot*128+j]
    pwT = np.ascontiguousarray(proj_w.T).astype(np.float32)
    pw = pwT.reshape(CT, 128, CT, 128)
    pwf = np.ascontiguousarray(pw.transpose(1, 2, 0, 3).reshape(128, CT * 768))
    bf = ml_dtypes.bfloat16
    return wqk0.astype(bf), wqf.astype(bf), wkf.astype(bf), wvc.astype(bf), pwf.astype(bf)


def kernel(x, mask, qkv_w, q_bias, v_bias, proj_w, proj_b, **_):
    x = np.asarray(x, np.float32)
    mask = np.asarray(mask)
    qkv_w = np.asarray(qkv_w, np.float32)
    q_bias = np.asarray(q_bias, np.float32)
    v_bias = np.asarray(v_bias, np.float32)
    proj_w = np.asarray(proj_w, np.float32)
    proj_b = np.asarray(proj_b, np.float32)

    if int((~mask).sum(axis=1).max()) > NK:
        return _kernel_numpy(x, mask, qkv_w, q_bias, v_bias, proj_w, proj_b)

    wqk0, wqf, wkf, wvc, pwf = _pack_weights(qkv_w, proj_w)
    qb = (q_bias * SCALE).astype(np.float32)
    pb_eff = (proj_b + proj_w @ v_bias).astype(np.float32)

    if "nc" not in _CACHE:
        _CACHE["nc"] = _build_nc()
    nc = _CACHE["nc"]

    in_maps = []
    for b in range(B):
        idx = np.flatnonzero(~mask[b])
        nk = len(idx)
        xkT = np.zeros((C, NK), np.float32)
        xkT[:, :nk] = x[b][idx].T
        mb = np.full(NK, MASK_NEG, np.float32)
        mb[:nk] = 0.0
        xTr = (
            np.ascontiguousarray(x[b].T)
            .reshape(CT, 128, N)
            .transpose(1, 0, 2)
            .reshape(128, CT * N)
        )
        xkr = xkT.reshape(CT, 128, NK).transpose(1, 0, 2).reshape(128, CT * NK)
        in_maps.append(
            {
                "xTr": np.ascontiguousarray(xTr).astype(ml_dtypes.bfloat16),
                "xkr": np.ascontiguousarray(xkr).astype(ml_dtypes.bfloat16),
                "wqk0": wqk0,
                "wqf": wqf,
                "wkf": wkf,
                "wvc": wvc,
                "pwf": pwf,
                "qb": qb,
                "mb": mb,
                "pb": pb_eff,
            }
        )

    _CACHE["last_in_maps"] = in_maps
    res = run_bass_kernel_spmd(nc, in_maps, list(range(B)))
    out = np.stack(
        [res.results[b]["out"].astype(np.float32).T for b in range(B)], axis=0
    )
    return np.ascontiguousarray(out)


if __name__ == "__main__":
    np.random.seed(0)
    x = np.random.randn(B, N, C).astype(np.float32)
    mask = np.random.randint(0, 2, (B, N)) > 0
    qkv_w = (np.random.randn(F3, C) * 0.02).astype(np.float32)
    q_bias = (np.random.randn(C) * 0.02).astype(np.float32)
    v_bias = (np.random.randn(C) * 0.02).astype(np.float32)
    proj_w = (np.random.randn(C, C) * 0.02).astype(np.float32)
    proj_b = (np.random.randn(C) * 0.02).astype(np.float32)
    out = kernel(x, mask, qkv_w, q_bias, v_bias, proj_w, proj_b)
    ref = _kernel_numpy(x, mask, qkv_w, q_bias, v_bias, proj_w, proj_b)
    rel = np.linalg.norm(out - ref) / np.linalg.norm(ref)
    print(out.shape, out.dtype, "rel err vs numpy:", rel)
